# revision 1
# baseline (speedup 1.0000x reference)
"""TRN2 Bass kernel for nn_DecoderRNN (ONLSTM decoder with additive attention).

Strategy (8 NeuronCores, SPMD — one program, per-core data):
  - Recurrence: batch-sharded, B=16 rows per core, 27 sequential steps.
    All recurrent state kept transposed [feature-on-partitions, batch-on-free]
    so every matmul runs weights-stationary with the tiny batch streaming.
  - Output projection: row-sharded — each core does its own 432 = 27*16 rows
    x full 30000 vocab, streaming out_W (pre-tiled bf16) from HBM. No
    collectives anywhere.
  - log_softmax without max-subtraction (logits are O(0.3)): per-row
    S = sum(exp(logit)); lp = Ln(expz * (1/S)) fused on the scalar engine.
  - All matmuls bf16 inputs with fp32 PSUM accumulation; elementwise and
    state math fp32.
"""
import numpy as np
import ml_dtypes

import concourse.bass as bass
import concourse.bacc as bacc
import concourse.mybir as mybir
from concourse.tile import TileContext
from concourse.masks import make_identity
from concourse.bass import IndirectOffsetOnAxis
from concourse.bass_utils import run_bass_kernel_spmd

F32 = mybir.dt.float32
BF16 = mybir.dt.bfloat16
I32 = mybir.dt.int32
AF = mybir.ActivationFunctionType
ALU = mybir.AluOpType
AX = mybir.AxisListType
BF = ml_dtypes.bfloat16

# dims
V, T, H, DW, PP, NCH, CH = 30000, 28, 512, 512, 256, 16, 32
B, SV, SP = 128, 40, 28
BC = 16              # batch per core
NS = T - 1           # 27 steps
ROWS = NS * BC       # 432
NVC = 59             # vocab chunks of 512 (pad 30000 -> 30208)
VPAD = NVC * 512
HDC = H // 128       # 4
PDC = PP // 128      # 2
NGT = 16             # gate tiles of 128 (2048 gate cols)
NM = 4               # row M-tiles in projection
M_ROWS = [128, 128, 128, 48]
SBUF_M = 2           # expz m-tiles kept in SBUF; rest spilled to DRAM


def _build(flags):
    nc = bacc.Bacc(None, target_bir_lowering=False)

    def din(name, shape, dtype):
        return nc.dram_tensor(name, list(shape), dtype, kind="ExternalInput")

    emb_d = din("emb", (V, DW), F32)
    idx_d = din("idx", (ROWS,), I32)
    encvT_d = din("encvT", (HDC, 128, SV * BC), F32)
    encvTb_d = din("encvTb", (HDC, 128, SV * BC), BF16)
    encpT_d = din("encpT", (PDC, 128, SP * BC), F32)
    encpTb_d = din("encpTb", (PDC, 128, SP * BC), BF16)
    Wah_d = din("Wah", (8, 128, 768), BF16)
    avWe_d = din("avWe", (HDC, 128, H), BF16)
    apWe_d = din("apWe", (PDC, 128, PP), BF16)
    w2v_d = din("w2v", (HDC, 128, 1), BF16)
    w2p_d = din("w2p", (PDC, 128, 1), BF16)
    b1v_d = din("b1v", (HDC, 128, 1), F32)
    b1p_d = din("b1p", (PDC, 128, 1), F32)
    ihW0x_d = din("ihW0x", (HDC, 128, 2048), BF16)
    ihW0xm_d = din("ihW0xm", (HDC, 128, 32), BF16)
    ihW0c_d = din("ihW0c", (HDC, 128, 2048), BF16)
    ihW0cm_d = din("ihW0cm", (HDC, 128, 32), BF16)
    hhW0_d = din("hhW0", (HDC, 128, 2048), BF16)
    hhW0m_d = din("hhW0m", (HDC, 128, 32), BF16)
    ihW1_d = din("ihW1", (HDC, 128, 2048), BF16)
    ihW1m_d = din("ihW1m", (HDC, 128, 32), BF16)
    hhW1_d = din("hhW1", (HDC, 128, 2048), BF16)
    hhW1m_d = din("hhW1m", (HDC, 128, 32), BF16)
    phW0_d = din("phW0", (PDC, 128, 32), BF16)
    phW1_d = din("phW1", (PDC, 128, 32), BF16)
    bg0_d = din("bg0", (128, NGT), F32)
    bg1_d = din("bg1", (128, NGT), F32)
    bm0_d = din("bm0", (1, 32), F32)
    bm1_d = din("bm1", (1, 32), F32)
    Ecin_d = din("Ecin", (HDC, 32, 128), F32)
    Ecf_d = din("Ecf", (HDC, 32, 128), F32)
    L32_d = din("L32", (32, 32), F32)
    E2_d = din("E2", (2, 32), F32)
    E2T_d = din("E2T", (32, 2), F32)
    outW_d = din("outW", (NVC, HDC, 128, 512), BF16)

    out_d = nc.dram_tensor("out", [ROWS, VPAD], F32, kind="ExternalOutput")
    spill_d = [
        nc.dram_tensor(f"spill{m}", [M_ROWS[m], VPAD], BF16, kind="Internal")
        for m in range(SBUF_M, NM)
    ]

    with TileContext(nc) as tc:
        with (
            tc.tile_pool(name="consts", bufs=1) as consts,
            tc.tile_pool(name="keep", bufs=1) as keep,
        ):
            # ---------------- constants ----------------
            id_bf = consts.tile([128, 128], BF16)
            make_identity(nc, id_bf)
            ones_bf = consts.tile([1, 128], BF16)
            nc.gpsimd.memset(ones_bf, 1.0)
            ones_f = consts.tile([1, ROWS], F32)
            nc.gpsimd.memset(ones_f, 1.0)
            Ecin = consts.tile([32, HDC, 128], F32)
            Ecf = consts.tile([32, HDC, 128], F32)
            for c in range(HDC):
                nc.sync.dma_start(out=Ecin[:, c], in_=Ecin_d[c])
                nc.sync.dma_start(out=Ecf[:, c], in_=Ecf_d[c])
            L32 = consts.tile([32, 32], F32)
            nc.sync.dma_start(out=L32, in_=L32_d[:, :])
            E2 = consts.tile([2, 32], F32)
            nc.sync.dma_start(out=E2, in_=E2_d[:, :])
            E2T = consts.tile([32, 2], F32)
            nc.sync.dma_start(out=E2T, in_=E2T_d[:, :])
            bg0 = consts.tile([128, NGT], F32)
            bg1 = consts.tile([128, NGT], F32)
            nc.sync.dma_start(out=bg0, in_=bg0_d[:, :])
            nc.sync.dma_start(out=bg1, in_=bg1_d[:, :])
            bm0 = consts.tile([1, 32], F32)
            bm1 = consts.tile([1, 32], F32)
            nc.sync.dma_start(out=bm0, in_=bm0_d[:, :])
            nc.sync.dma_start(out=bm1, in_=bm1_d[:, :])
            w2v = consts.tile([128, HDC, 1], BF16)
            w2p = consts.tile([128, PDC, 1], BF16)
            b1v = consts.tile([128, HDC, 1], F32)
            b1p = consts.tile([128, PDC, 1], F32)
            for c in range(HDC):
                nc.sync.dma_start(out=w2v[:, c], in_=w2v_d[c])
                nc.sync.dma_start(out=b1v[:, c], in_=b1v_d[c])
            for c in range(PDC):
                nc.sync.dma_start(out=w2p[:, c], in_=w2p_d[c])
                nc.sync.dma_start(out=b1p[:, c], in_=b1p_d[c])

            # h1 for all steps (bf16) — projection lhsT
            h1_all = keep.tile([128, HDC, NS, BC], BF16)

            if flags.get("skip_recur"):
                nc.gpsimd.memset(h1_all, 0.0)
            # ================= recurrence scope =================
            if not flags.get("skip_recur"):
                with (
                    tc.tile_pool(name="rkeep", bufs=1) as rk,
                    tc.tile_pool(name="states", bufs=3) as stp,
                    tc.tile_pool(name="wk", bufs=2) as wk,
                    tc.tile_pool(name="wkbig", bufs=1) as wkb,
                ):
                    def wload(pool, dram, kdim, n, nm, dt=BF16):
                        t = pool.tile([128, kdim, n], dt, name=nm, tag=nm, bufs=1)
                        for c in range(kdim):
                            nc.sync.dma_start(out=t[:, c], in_=dram[c])
                        return t

                    encvTb = rk.tile([128, HDC, SV * BC], BF16)
                    encpTb = rk.tile([128, PDC, SP * BC], BF16)
                    for c in range(HDC):
                        nc.sync.dma_start(out=encvTb[:, c], in_=encvTb_d[c])
                    for c in range(PDC):
                        nc.sync.dma_start(out=encpTb[:, c], in_=encpTb_d[c])

                    stat0 = rk.tile([128, NGT, ROWS], BF16)
                    m0stat = rk.tile([32, NS, BC], F32)
                    encWv = rk.tile([128, HDC, SV * BC], BF16)
                    encWp = rk.tile([128, PDC, SP * BC], BF16)

                    # ---- preamble ----
                    with (
                        tc.tile_pool(name="pre", bufs=2) as pre,
                        tc.tile_pool(name="ppre", bufs=1, space="PSUM") as ppre,
                    ):
                        avWe = wload(pre, avWe_d, HDC, H, "avWe")
                        apWe = wload(pre, apWe_d, PDC, PP, "apWe")
                        ihW0x = wload(pre, ihW0x_d, HDC, 2048, "ihW0x")
                        ihW0xm = wload(pre, ihW0xm_d, HDC, 32, "ihW0xm")

                        NTI = (ROWS + 127) // 128
                        idx_sb = pre.tile([128, NTI], I32, tag="idx")
                        nfull = ROWS // 128
                        if nfull:
                            nc.sync.dma_start(
                                out=idx_sb[:, :nfull],
                                in_=idx_d[: nfull * 128].rearrange("(i p) -> p i", p=128),
                            )
                        if ROWS % 128:
                            nc.sync.dma_start(
                                out=idx_sb[: ROWS % 128, nfull : nfull + 1],
                                in_=idx_d[nfull * 128 :],
                            )
                        embT = pre.tile([128, HDC, ROWS], BF16, tag="embT")
                        for i in range(NTI):
                            n = min(128, ROWS - i * 128)
                            esb = pre.tile([128, DW], F32, tag="esb")
                            nc.gpsimd.indirect_dma_start(
                                out=esb[:n],
                                out_offset=None,
                                in_=emb_d[:, :],
                                in_offset=IndirectOffsetOnAxis(
                                    ap=idx_sb[:n, i : i + 1], axis=0
                                ),
                            )
                            ebf = pre.tile([128, DW], BF16, tag="ebf")
                            nc.vector.tensor_copy(out=ebf[:n], in_=esb[:n])
                            for c in range(HDC):
                                tp = ppre.tile([128, 128], BF16, tag="tp")
                                nc.tensor.transpose(
                                    tp[:, :n],
                                    ebf[:n, c * 128 : (c + 1) * 128],
                                    id_bf[:n, :n],
                                )
                                nc.vector.tensor_copy(
                                    out=embT[:, c, i * 128 : i * 128 + n], in_=tp[:, :n]
                                )

                        # static gate part from xt: stat0 = ihW0x.T @ embT (+bias_g0)
                        for gt in range(NGT):
                            sp = ppre.tile([128, ROWS], F32, tag="sp")
                            for c in range(HDC):
                                nc.tensor.matmul(
                                    sp,
                                    ihW0x[:, c, gt * 128 : (gt + 1) * 128],
                                    embT[:, c],
                                    start=(c == 0),
                                    stop=(c == HDC - 1),
                                )
                            if flags["bg0_nz"]:
                                nc.vector.tensor_tensor(
                                    out=stat0[:, gt], in0=sp,
                                    in1=bg0[:, gt : gt + 1].to_broadcast([128, ROWS]),
                                    op=ALU.add,
                                )
                            else:
                                nc.vector.tensor_copy(out=stat0[:, gt], in_=sp)
                        # static master part (transposed): ihW0xm.T @ embT + bm0
                        mp = ppre.tile([32, ROWS], F32, tag="mp")
                        for c in range(HDC):
                            nc.tensor.matmul(
                                mp,
                                ihW0xm[:, c],
                                embT[:, c],
                                start=(c == 0),
                                stop=(c == HDC - 1 and not flags["bm0_nz"]),
                            )
                        if flags["bm0_nz"]:
                            nc.tensor.matmul(mp, bm0, ones_f, start=False, stop=True)
                        nc.vector.tensor_copy(
                            out=m0stat.rearrange("p t b -> p (t b)"), in_=mp
                        )

                        # encoder attention precompute (enc @ W1_enc + b1), transposed
                        for m in range(HDC):
                            ep = ppre.tile([128, 2, 512], F32, tag="ep")
                            for hh in range(2):
                                for c in range(HDC):
                                    nc.tensor.matmul(
                                        ep[:, hh, :320],
                                        avWe[:, c, m * 128 : (m + 1) * 128],
                                        encvTb[:, c, hh * 320 : (hh + 1) * 320],
                                        start=(c == 0),
                                        stop=(c == HDC - 1),
                                    )
                                if flags["b1v_nz"]:
                                    nc.vector.tensor_tensor(
                                        out=encWv[:, m, hh * 320 : (hh + 1) * 320],
                                        in0=ep[:, hh, :320],
                                        in1=b1v[:, m].to_broadcast([128, 320]),
                                        op=ALU.add,
                                    )
                                else:
                                    nc.vector.tensor_copy(
                                        out=encWv[:, m, hh * 320 : (hh + 1) * 320],
                                        in_=ep[:, hh, :320],
                                    )
                        for m in range(PDC):
                            ep2 = ppre.tile([128, SP * BC], F32, tag="ep2")
                            for c in range(PDC):
                                nc.tensor.matmul(
                                    ep2,
                                    apWe[:, c, m * 128 : (m + 1) * 128],
                                    encpTb[:, c],
                                    start=(c == 0),
                                    stop=(c == PDC - 1),
                                )
                            if flags["b1p_nz"]:
                                nc.vector.tensor_tensor(
                                    out=encWp[:, m], in0=ep2,
                                    in1=b1p[:, m].to_broadcast([128, SP * BC]),
                                    op=ALU.add,
                                )
                            else:
                                nc.vector.tensor_copy(out=encWp[:, m], in_=ep2)

                    # ---- states ----
                    h0T = stp.tile([128, HDC, BC], F32, tag="hn0")
                    c0T = stp.tile([128, HDC, BC], F32, tag="cn0")
                    h1T = stp.tile([128, HDC, BC], F32, tag="hn1")
                    c1T = stp.tile([128, HDC, BC], F32, tag="cn1")
                    for s in (h0T, c0T, h1T, c1T):
                        nc.gpsimd.memset(s, 0.0)

                    with (
                        tc.tile_pool(name="wpool", bufs=1) as wp,
                        tc.tile_pool(name="pstep", bufs=1, space="PSUM") as pst,
                    ):
                        Wah = wload(wp, Wah_d, 8, 768, "Wah")
                        ihW0c = wload(wp, ihW0c_d, HDC, 2048, "ihW0c")
                        hhW0 = wload(wp, hhW0_d, HDC, 2048, "hhW0")
                        ihW1 = wload(wp, ihW1_d, HDC, 2048, "ihW1")
                        hhW1 = wload(wp, hhW1_d, HDC, 2048, "hhW1")
                        ihW0cm = wload(wp, ihW0cm_d, HDC, 32, "ihW0cm")
                        hhW0m = wload(wp, hhW0m_d, HDC, 32, "hhW0m")
                        ihW1m = wload(wp, ihW1m_d, HDC, 32, "ihW1m")
                        hhW1m = wload(wp, hhW1m_d, HDC, 32, "hhW1m")
                        phW0 = wload(wp, phW0_d, PDC, 32, "phW0")
                        phW1 = wload(wp, phW1_d, PDC, 32, "phW1")

                        def attend(hidS, hid_off, ndc, S, encWb, encTb, w2, tag):
                            nb = S * BC
                            nh = (nb + 511) // 512
                            half = (nb + nh - 1) // nh
                            tz = wkb.tile([128, ndc, nb], BF16, tag=f"tz{tag}")
                            for c in range(ndc):
                                for hh in range(nh):
                                    lo, hi = hh * half, min((hh + 1) * half, nb)
                                    ns = (hi - lo) // BC
                                    zc = pst.tile([128, 512], F32, tag="z", bufs=2)
                                    nc.tensor.matmul(
                                        zc[:, : hi - lo], id_bf,
                                        encWb[:, c, lo:hi], start=True, stop=False,
                                    )
                                    nc.tensor.matmul(
                                        zc[:, : hi - lo].rearrange(
                                            "p (s b) -> p s b", b=BC),
                                        id_bf,
                                        hidS[:, hid_off + c]
                                        .rearrange("p b -> p () b")
                                        .to_broadcast([128, ns, BC]),
                                        start=False, stop=True,
                                    )
                                    nc.scalar.activation(
                                        tz[:, c, lo:hi], zc[:, : hi - lo], AF.Tanh
                                    )
                            e_ps = pst.tile([1, nh, 512], F32, tag="e")
                            for hh in range(nh):
                                lo, hi = hh * half, min((hh + 1) * half, nb)
                                for c in range(ndc):
                                    nc.tensor.matmul(
                                        e_ps[:, hh, : hi - lo],
                                        w2[:, c],
                                        tz[:, c, lo:hi],
                                        start=(c == 0),
                                        stop=(c == ndc - 1),
                                    )
                            aexp = wk.tile([1, nb], BF16, tag=f"ax{tag}")
                            nc.scalar.activation(
                                aexp.rearrange("o (h x) -> o h x", h=nh),
                                e_ps[:, :, :half],
                                AF.Exp,
                            )
                            # unnormalized context; 1/sum folded in at the end
                            ssum = wk.tile([1, BC], F32, tag=f"ss{tag}")
                            nc.vector.tensor_reduce(
                                out=ssum,
                                in_=aexp.rearrange("o (s b) -> o b s", b=BC),
                                axis=AX.X,
                                op=ALU.add,
                            )
                            rec = wk.tile([1, BC], F32, tag=f"rc{tag}")
                            nc.vector.reciprocal(rec, ssum)
                            rrep = pst.tile([128, 512], F32, tag="z", bufs=2)
                            nc.tensor.matmul(
                                rrep[:, :BC], ones_f[:, :128], rec,
                                start=True, stop=True,
                            )
                            arep = pst.tile([128, nh, 512], F32, tag="e")
                            for hh in range(nh):
                                lo, hi = hh * half, min((hh + 1) * half, nb)
                                nc.tensor.matmul(
                                    arep[:, hh, : hi - lo],
                                    ones_bf,
                                    aexp[:, lo:hi],
                                    start=True,
                                    stop=True,
                                )
                            cvT = wk.tile([128, ndc, BC], F32, tag=f"cv{tag}")
                            prod = wkb.tile([128, ndc, nb], F32, tag=f"pr{tag}")
                            for hh in range(nh):
                                lo, hi = hh * half, min((hh + 1) * half, nb)
                                nc.vector.tensor_tensor(
                                    out=prod[:, :, lo:hi],
                                    in0=encTb[:, :, lo:hi],
                                    in1=arep[:, hh, : hi - lo]
                                    .rearrange("p x -> p () x")
                                    .to_broadcast([128, ndc, hi - lo]),
                                    op=ALU.mult,
                                )
                            nc.vector.tensor_reduce(
                                out=cvT,
                                in_=prod.rearrange("p c (s b) -> p c b s", b=BC),
                                axis=AX.X,
                                op=ALU.add,
                            )
                            cvb = wk.tile([128, ndc, BC], BF16, tag=f"cb{tag}")
                            nc.vector.tensor_tensor(
                                out=cvb,
                                in0=cvT,
                                in1=rrep[:, :BC]
                                .rearrange("p b -> p () b")
                                .to_broadcast([128, ndc, BC]),
                                op=ALU.mult,
                            )
                            return cvb

                        def cumsoft_reps(mch, m_ps, mstat_ap, tag):
                            # m_ps: psum [32, BC] master logits (transposed)
                            if mstat_ap is not None:
                                ms = wk.tile([32, BC], F32, tag=f"ms{tag}")
                                nc.vector.tensor_tensor(
                                    out=ms, in0=m_ps, in1=mstat_ap, op=ALU.add
                                )
                                esrc = ms
                            else:
                                esrc = m_ps
                            em = wk.tile([32, BC], F32, tag=f"em{tag}")
                            nc.scalar.activation(em, esrc, AF.Exp)
                            cs = mch[:32, 2 * BC : 3 * BC]
                            nc.tensor.matmul(cs, L32, em, start=True, stop=True)
                            tot = mch[:2, 18 * BC : 19 * BC]
                            nc.tensor.matmul(tot, E2T, em, start=True, stop=True)
                            rec2 = wk.tile([2, BC], F32, tag=f"r2{tag}")
                            nc.vector.reciprocal(rec2, tot)
                            rr = mch[:32, 3 * BC : 4 * BC]
                            nc.tensor.matmul(rr, E2, rec2, start=True, stop=True)
                            rrS = wk.tile([32, BC], F32, tag=f"rrS{tag}")
                            nc.vector.tensor_copy(out=rrS, in_=rr)
                            csn = wk.tile([32, BC], F32, tag=f"cf{tag}")
                            nc.vector.tensor_tensor(
                                out=csn, in0=cs, in1=rrS, op=ALU.mult
                            )
                            ci32 = wk.tile([32, BC], F32, tag=f"ci{tag}")
                            nc.vector.tensor_scalar(
                                out=ci32, in0=csn, scalar1=-1.0, scalar2=1.0,
                                op0=ALU.mult, op1=ALU.add,
                            )
                            rep = mch[:, 4 * BC : 4 * BC + HDC * 2 * BC].rearrange(
                                "p (c a b) -> p c a b", c=HDC, a=2
                            )
                            for tau in range(HDC):
                                nc.tensor.matmul(
                                    rep[:, tau, 0], Ecin[:, tau], ci32,
                                    start=True, stop=True,
                                )
                                nc.tensor.matmul(
                                    rep[:, tau, 1], Ecf[:, tau], csn,
                                    start=True, stop=True,
                                )
                            repS = wk.tile([128, HDC, 2, BC], F32, tag=f"rs{tag}")
                            nc.vector.tensor_copy(out=repS, in_=rep)
                            return repS

                        def combine(ga, repS, cT, tag):
                            ci = repS[:, :, 0]
                            cf = repS[:, :, 1]
                            ov = wk.tile([128, HDC, BC], F32, tag=f"ov{tag}")
                            nc.vector.tensor_tensor(out=ov, in0=ci, in1=cf, op=ALU.mult)
                            fg_ = wk.tile([128, HDC, BC], F32, tag=f"fg{tag}")
                            ig_ = wk.tile([128, HDC, BC], F32, tag=f"ig{tag}")
                            tmp = wk.tile([128, HDC, BC], F32, tag=f"tm{tag}")
                            nc.vector.tensor_tensor(
                                out=fg_, in0=ga[:, 8:12], in1=ov, op=ALU.mult
                            )
                            nc.vector.tensor_tensor(
                                out=tmp, in0=cf, in1=ov, op=ALU.subtract
                            )
                            nc.vector.tensor_tensor(out=fg_, in0=fg_, in1=tmp, op=ALU.add)
                            nc.vector.tensor_tensor(
                                out=ig_, in0=ga[:, 4:8], in1=ov, op=ALU.mult
                            )
                            nc.vector.tensor_tensor(
                                out=tmp, in0=ci, in1=ov, op=ALU.subtract
                            )
                            nc.vector.tensor_tensor(out=ig_, in0=ig_, in1=tmp, op=ALU.add)
                            cn = stp.tile([128, HDC, BC], F32, tag=f"cn{tag}")
                            nc.vector.tensor_tensor(out=cn, in0=fg_, in1=cT, op=ALU.mult)
                            nc.vector.tensor_tensor(
                                out=tmp, in0=ig_, in1=ga[:, 12:16], op=ALU.mult
                            )
                            nc.vector.tensor_tensor(out=cn, in0=cn, in1=tmp, op=ALU.add)
                            tcy = wk.tile([128, HDC, BC], F32, tag=f"tc{tag}")
                            nc.scalar.activation(tcy, cn, AF.Tanh)
                            hn = stp.tile([128, HDC, BC], F32, tag=f"hn{tag}")
                            nc.vector.tensor_tensor(
                                out=hn, in0=ga[:, 0:4], in1=tcy, op=ALU.mult
                            )
                            return hn, cn

                        # ================= the 27 steps =================
                        MO_HID = 192          # free-offset of hid inside mch
                        for t in range(NS):
                            h0b = wk.tile([128, HDC, BC], BF16, tag="h0b")
                            h1b = wk.tile([128, HDC, BC], BF16, tag="h1b")
                            nc.vector.tensor_copy(out=h0b, in_=h0T)
                            nc.vector.tensor_copy(out=h1b, in_=h1T)
                            mch = pst.tile([128, 512], F32, tag="mch")
                            hid = mch[:, MO_HID : MO_HID + 6 * BC].rearrange(
                                "p (m b) -> p m b", b=BC
                            )
                            m0 = mch[:32, 0:BC]
                            m1 = mch[:32, BC : 2 * BC]
                            for mt in range(6):
                                for kc in range(8):
                                    rhs = h0b[:, kc] if kc < 4 else h1b[:, kc - 4]
                                    nc.tensor.matmul(
                                        hid[:, mt],
                                        Wah[:, kc, mt * 128 : (mt + 1) * 128],
                                        rhs,
                                        start=(kc == 0),
                                        stop=(kc == 7),
                                    )
                            # gate/master matmuls that depend only on prior state
                            # start their PSUM groups early (overlap attention)
                            g0 = pst.tile([128, NGT, BC], F32, tag="g", bufs=2)
                            g1 = pst.tile([128, NGT, BC], F32, tag="g", bufs=2)
                            for gt in range(NGT):
                                for kc in range(HDC):
                                    nc.tensor.matmul(
                                        g0[:, gt],
                                        hhW0[:, kc, gt * 128 : (gt + 1) * 128],
                                        h0b[:, kc],
                                        start=(kc == 0),
                                        stop=False,
                                    )
                            for gt in range(NGT):
                                for kc in range(HDC):
                                    nc.tensor.matmul(
                                        g1[:, gt],
                                        hhW1[:, kc, gt * 128 : (gt + 1) * 128],
                                        h1b[:, kc],
                                        start=(kc == 0),
                                        stop=False,
                                    )
                            for kc in range(HDC):
                                nc.tensor.matmul(
                                    m0, hhW0m[:, kc], h0b[:, kc],
                                    start=(kc == 0), stop=False,
                                )
                            for kc in range(HDC):
                                nc.tensor.matmul(
                                    m1, hhW1m[:, kc], h1b[:, kc],
                                    start=(kc == 0), stop=False,
                                )
                            hidS = wk.tile([128, 6, BC], BF16, tag="hidS")
                            nc.vector.tensor_copy(out=hidS, in_=hid)
                            cvb = attend(hidS, 0, HDC, SV, encWv, encvTb, w2v, "v")
                            cpb = attend(hidS, 4, PDC, SP, encWp, encpTb, w2p, "p")

                            # ---- layer 0 finish ----
                            for kc in range(PDC):
                                nc.tensor.matmul(
                                    m0, phW0[:, kc], cpb[:, kc],
                                    start=False, stop=False,
                                )
                                nc.tensor.matmul(
                                    m1, phW1[:, kc], cpb[:, kc],
                                    start=False, stop=False,
                                )
                            for kc in range(HDC):
                                nc.tensor.matmul(
                                    m0, ihW0cm[:, kc], cvb[:, kc],
                                    start=False, stop=(kc == HDC - 1),
                                )
                            for gt in range(NGT):
                                for kc in range(HDC):
                                    nc.tensor.matmul(
                                        g0[:, gt],
                                        ihW0c[:, kc, gt * 128 : (gt + 1) * 128],
                                        cvb[:, kc],
                                        start=False,
                                        stop=(kc == HDC - 1),
                                    )
                            rep0 = cumsoft_reps(mch, m0, m0stat[:, t], "0")
                            g0s = wk.tile([128, NGT, BC], F32, tag="g0s")
                            nc.vector.tensor_tensor(
                                out=g0s,
                                in0=g0,
                                in1=stat0.rearrange("p g (t b) -> p g t b", b=BC)[:, :, t],
                                op=ALU.add,
                            )
                            g0a = wk.tile([128, NGT, BC], F32, tag="g0a")
                            g0t = wk.tile([128, 12, BC], F32, tag="g0t")
                            nc.scalar.activation(g0t, g0s[:, 0:12], AF.Tanh, scale=0.5)
                            nc.scalar.activation(g0a[:, 12:16], g0s[:, 12:16], AF.Tanh)
                            nc.vector.tensor_scalar(
                                out=g0a[:, 0:12], in0=g0t, scalar1=0.5,
                                scalar2=0.5, op0=ALU.mult, op1=ALU.add)
                            h0T, c0T = combine(g0a, rep0, c0T, "0")

                            # ---- layer 1 finish ----
                            h0b2 = wk.tile([128, HDC, BC], BF16, tag="h0b2")
                            nc.vector.tensor_copy(out=h0b2, in_=h0T)
                            for kc in range(HDC):
                                nc.tensor.matmul(
                                    m1, ihW1m[:, kc], h0b2[:, kc],
                                    start=False,
                                    stop=(kc == HDC - 1 and not flags["bm1_nz"]),
                                )
                            if flags["bm1_nz"]:
                                nc.tensor.matmul(
                                    m1, bm1, ones_f[:, :BC], start=False, stop=True
                                )
                            for gt in range(NGT):
                                for kc in range(HDC):
                                    nc.tensor.matmul(
                                        g1[:, gt],
                                        ihW1[:, kc, gt * 128 : (gt + 1) * 128],
                                        h0b2[:, kc],
                                        start=False,
                                        stop=(kc == HDC - 1),
                                    )
                            rep1 = cumsoft_reps(mch, m1, None, "1")
                            g1a = wk.tile([128, NGT, BC], F32, tag="g1a")
                            if flags["bg1_nz"]:
                                g1s = wk.tile([128, NGT, BC], F32, tag="g1s")
                                for gt in range(NGT):
                                    nc.vector.tensor_tensor(
                                        out=g1s[:, gt],
                                        in0=g1[:, gt],
                                        in1=bg1[:, gt : gt + 1].to_broadcast([128, BC]),
                                        op=ALU.add,
                                    )
                                gsrc = g1s
                            else:
                                gsrc = g1
                            g1t = wk.tile([128, 12, BC], F32, tag="g1t")
                            nc.scalar.activation(g1t, gsrc[:, 0:12], AF.Tanh, scale=0.5)
                            nc.scalar.activation(g1a[:, 12:16], gsrc[:, 12:16], AF.Tanh)
                            nc.vector.tensor_scalar(
                                out=g1a[:, 0:12], in0=g1t, scalar1=0.5,
                                scalar2=0.5, op0=ALU.mult, op1=ALU.add)
                            h1T, c1T = combine(g1a, rep1, c1T, "1")
                            nc.vector.tensor_copy(out=h1_all[:, :, t], in_=h1T)

            # ================= projection =================
            if not flags.get("skip_proj"):
                with (
                    tc.tile_pool(name="pj", bufs=1) as pj,
                    tc.tile_pool(name="wst", bufs=12) as wst,
                    tc.tile_pool(name="lpout", bufs=2) as lpo,
                    tc.tile_pool(name="pproj", bufs=4, space="PSUM") as ppj,
                ):
                    expz_sb = [
                        pj.tile([128, NVC, 512], BF16, tag=f"ez{m}", name=f"ez{m}")
                        for m in range(SBUF_M)
                    ]
                    Sacc = pj.tile([128, NM, NVC], F32)
                    nc.gpsimd.memset(Sacc, 0.0)
                    h1f = h1_all.rearrange("p c t b -> p c (t b)")
                    dma_engs = [nc.sync, nc.scalar]
                    for v in range(NVC):
                        wts = []
                        for kc in range(HDC):
                            wt = wst.tile([128, 512], BF16, tag="wt")
                            nc.sync.dma_start(out=wt, in_=outW_d[v, kc])
                            wts.append(wt)
                        for m in range(NM):
                            nr = M_ROWS[m]
                            ps = ppj.tile([128, 512], F32, tag="ps")
                            for kc in range(HDC):
                                nc.tensor.matmul(
                                    ps[:nr],
                                    h1f[:, kc, m * 128 : m * 128 + nr],
                                    wts[kc],
                                    start=(kc == 0),
                                    stop=(kc == HDC - 1),
                                )
                            if m < SBUF_M:
                                nc.scalar.activation(
                                    expz_sb[m][:, v, :],
                                    ps[:nr],
                                    AF.Exp,
                                    accum_out=Sacc[:nr, m, v : v + 1],
                                )
                            else:
                                ezt = lpo.tile([128, 512], BF16, tag="ezs")
                                nc.scalar.activation(
                                    ezt[:nr],
                                    ps[:nr],
                                    AF.Exp,
                                    accum_out=Sacc[:nr, m, v : v + 1],
                                )
                                nc.gpsimd.dma_start(
                                    out=spill_d[m - SBUF_M][:, v * 512 : (v + 1) * 512],
                                    in_=ezt[:nr],
                                )
                    recS = pj.tile([128, NM], F32)
                    Stot = pj.tile([128, NM], F32)
                    nc.vector.tensor_reduce(out=Stot, in_=Sacc, axis=AX.X, op=ALU.add)
                    nc.vector.reciprocal(recS, Stot)
                    VB = 8  # pass-2 chunk: 8 vocab chunks of 512
                    for m in range(NM):
                        nr = M_ROWS[m]
                        for v0 in range(0, NVC, VB):
                            nv = min(VB, NVC - v0) * 512
                            if m < SBUF_M:
                                ez = expz_sb[m].rearrange("p v x -> p (v x)")[
                                    :nr, v0 * 512 : v0 * 512 + nv
                                ]
                            else:
                                ld = lpo.tile([128, VB * 512], BF16, tag="ld")
                                nc.sync.dma_start(
                                    out=ld[:nr, :nv],
                                    in_=spill_d[m - SBUF_M][:, v0 * 512 : v0 * 512 + nv],
                                )
                                ez = ld[:nr, :nv]
                            lp = lpo.tile([128, VB * 512], F32, tag="lp")
                            nc.scalar.activation(
                                lp[:nr, :nv], ez, AF.Ln, scale=recS[:nr, m : m + 1]
                            )
                            nc.sync.dma_start(
                                out=out_d[m * 128 : m * 128 + nr, v0 * 512 : v0 * 512 + nv],
                                in_=lp[:nr, :nv],
                            )



    nc.finalize()
    return nc


def _prep(inputs):
    """Host-side input prep: slicing/transposing/casting only."""
    f32 = np.float32
    g = {k: np.asarray(v) for k, v in inputs.items()}
    av_W1, ap_W1 = g["av_W1"].astype(f32), g["ap_W1"].astype(f32)
    shared = {}
    shared["emb"] = np.ascontiguousarray(g["embedding"].astype(f32))
    shared["Wah"] = np.ascontiguousarray(
        np.concatenate([av_W1[H:], ap_W1[PP:]], axis=1).reshape(8, 128, 768)
    ).astype(BF)
    shared["avWe"] = np.ascontiguousarray(av_W1[:H].reshape(HDC, 128, H)).astype(BF)
    shared["apWe"] = np.ascontiguousarray(ap_W1[:PP].reshape(PDC, 128, PP)).astype(BF)
    shared["w2v"] = g["av_w2"].astype(f32).reshape(HDC, 128, 1).astype(BF)
    shared["w2p"] = g["ap_w2"].astype(f32).reshape(PDC, 128, 1).astype(BF)
    shared["b1v"] = np.ascontiguousarray(g["av_b1"].astype(f32).reshape(HDC, 128, 1))
    shared["b1p"] = np.ascontiguousarray(g["ap_b1"].astype(f32).reshape(PDC, 128, 1))

    def gperm(Wg):
        # reference gate col order [outg|cellg|ing|fg] -> [outg|ing|fg|cellg]
        return np.concatenate(
            [Wg[..., 0:512], Wg[..., 1024:2048], Wg[..., 512:1024]], axis=-1)

    def cellw(W, kdim, pref):
        W = np.asarray(W, f32)
        return {
            pref: np.ascontiguousarray(
                gperm(W[:, 32:]).reshape(kdim, 128, 2048)).astype(BF),
            pref + "m": np.ascontiguousarray(W[:, :32].reshape(kdim, 128, 32)).astype(BF),
        }

    shared.update(cellw(g["ih_W0"][:DW], HDC, "ihW0x"))
    shared.update(cellw(g["ih_W0"][DW:], HDC, "ihW0c"))
    shared.update(cellw(g["hh_W0"], HDC, "hhW0"))
    shared.update(cellw(g["ih_W1"], HDC, "ihW1"))
    shared.update(cellw(g["hh_W1"], HDC, "hhW1"))
    shared["phW0"] = np.ascontiguousarray(
        g["ph_W0"].astype(f32).reshape(PDC, 128, 32)).astype(BF)
    shared["phW1"] = np.ascontiguousarray(
        g["ph_W1"].astype(f32).reshape(PDC, 128, 32)).astype(BF)
    bg0 = gperm((g["ih_b0"] + g["hh_b0"]).astype(f32)[32:])
    bg1 = gperm((g["ih_b1"] + g["hh_b1"]).astype(f32)[32:])
    shared["bg0"] = np.ascontiguousarray(bg0.reshape(NGT, 128).T)
    shared["bg1"] = np.ascontiguousarray(bg1.reshape(NGT, 128).T)
    bm0 = (g["ih_b0"][:32] + g["hh_b0"][:32] + g["ph_b0"]).astype(f32)
    bm1 = (g["ih_b1"][:32] + g["hh_b1"][:32] + g["ph_b1"]).astype(f32)
    shared["bm0"] = np.ascontiguousarray(bm0.reshape(1, 32))
    shared["bm1"] = np.ascontiguousarray(bm1.reshape(1, 32))
    Ecin = np.zeros((HDC, 32, 128), f32)
    Ecf = np.zeros((HDC, 32, 128), f32)
    for tau in range(HDC):
        for mcol in range(128):
            c = (tau * 128 + mcol) // CH
            Ecin[tau, c, mcol] = 1.0
            Ecf[tau, NCH + c, mcol] = 1.0
    shared["Ecin"] = Ecin
    shared["Ecf"] = Ecf
    L32 = np.zeros((32, 32), f32)
    for k in range(32):
        for m2 in range(32):
            if k // NCH == m2 // NCH and k % NCH <= m2 % NCH:
                L32[k, m2] = 1.0
    shared["L32"] = L32
    E2 = np.zeros((2, 32), f32)
    E2[0, :NCH] = 1.0
    E2[1, NCH:] = 1.0
    shared["E2"] = E2
    shared["E2T"] = np.ascontiguousarray(E2.T)
    oW = np.zeros((DW, VPAD), f32)
    oW[:, :V] = g["out_W"].astype(f32)
    shared["outW"] = np.ascontiguousarray(
        oW.reshape(HDC, 128, NVC, 512).transpose(2, 0, 1, 3)).astype(BF)

    flags = {
        "bg0_nz": bool(np.any(bg0 != 0)),
        "b1v_nz": bool(np.any(np.asarray(g["av_b1"]) != 0)),
        "b1p_nz": bool(np.any(np.asarray(g["ap_b1"]) != 0)),
        "bg1_nz": bool(np.any(bg1 != 0)),
        "bm0_nz": bool(np.any(bm0 != 0)),
        "bm1_nz": bool(np.any(bm1 != 0)),
        "outb_nz": bool(np.any(np.asarray(g["out_b"]) != 0)),
    }
    if flags["outb_nz"]:
        raise NotImplementedError("nonzero out_b path not wired")

    in_maps = []
    targets = np.asarray(g["targets"])
    enc_v = np.asarray(g["encoder_outputs"], f32)
    enc_p = np.asarray(g["encoder_outputs_parse"], f32)
    for r in range(8):
        m = dict(shared)
        sl = slice(BC * r, BC * (r + 1))
        m["idx"] = np.ascontiguousarray(
            targets[sl, :NS].T.reshape(-1).astype(np.int32))
        evT = np.ascontiguousarray(
            enc_v[sl].transpose(2, 1, 0).reshape(HDC, 128, SV * BC))
        epT = np.ascontiguousarray(
            enc_p[sl].transpose(2, 1, 0).reshape(PDC, 128, SP * BC))
        m["encvT"] = evT
        m["encvTb"] = evT.astype(BF)
        m["encpT"] = epT
        m["encpTb"] = epT.astype(BF)
        in_maps.append(m)
    return in_maps, flags


def kernel(**inputs):
    in_maps, flags = _prep(inputs)
    nc = _build(flags)
    res = run_bass_kernel_spmd(nc, in_maps, core_ids=list(range(8)))
    outs = []
    for r in range(8):
        o = np.asarray(res.results[r]["out"])[:, :V]      # (432, 30000)
        outs.append(o.reshape(NS, BC, V).transpose(1, 0, 2))
    return np.ascontiguousarray(np.concatenate(outs, axis=0).astype(np.float32))



# revision 9
# speedup vs baseline: 1.2210x; 1.2210x over previous
"""TRN2 Bass kernel for nn_DecoderRNN (ONLSTM decoder with additive attention).

Strategy (8 NeuronCores, SPMD — one program, per-core data):
  - Recurrence: batch-sharded, B=16 rows per core, 27 sequential steps.
    All recurrent state kept transposed [feature-on-partitions, batch-on-free]
    so every matmul runs weights-stationary with the tiny batch streaming.
  - Output projection: row-sharded — each core does its own 432 = 27*16 rows
    x full 30000 vocab, streaming out_W (pre-tiled bf16) from HBM. No
    collectives anywhere.
  - log_softmax without max-subtraction (logits are O(0.3)): per-row
    S = sum(exp(logit)); lp = Ln(expz * (1/S)) fused on the scalar engine.
  - All matmuls bf16 inputs with fp32 PSUM accumulation; elementwise and
    state math fp32.
"""
import numpy as np
import ml_dtypes

import concourse.bass as bass
import concourse.bacc as bacc
import concourse.mybir as mybir
from concourse.tile import TileContext
from concourse.masks import make_identity
from concourse.bass import IndirectOffsetOnAxis
from concourse.bass_utils import run_bass_kernel_spmd

F32 = mybir.dt.float32
BF16 = mybir.dt.bfloat16
I32 = mybir.dt.int32
AF = mybir.ActivationFunctionType
ALU = mybir.AluOpType
AX = mybir.AxisListType
BF = ml_dtypes.bfloat16

# dims
V, T, H, DW, PP, NCH, CH = 30000, 28, 512, 512, 256, 16, 32
B, SV, SP = 128, 40, 28
BC = 16              # batch per core
NS = T - 1           # 27 steps
ROWS = NS * BC       # 432
NVC = 59             # vocab chunks of 512 (pad 30000 -> 30208)
VPAD = NVC * 512
HDC = H // 128       # 4
PDC = PP // 128      # 2
NGT = 16             # gate tiles of 128 (2048 gate cols)
NM = 4               # row M-tiles in projection
M_ROWS = [128, 128, 128, 48]
FP8 = mybir.dt.float8e4
H1SC = 8.0           # h1 fp8 pre-scale
WSC = 16.0           # out_W fp8 pre-scale
ZSC = 1.0 / (H1SC * WSC)
AGRP = 3             # pass-A psum group: 3 vocab chunks (1536 f32 = 3 banks)
DR = mybir.MatmulPerfMode.DoubleRow


def _build(flags):
    nc = bacc.Bacc(None, target_bir_lowering=False)

    def din(name, shape, dtype):
        return nc.dram_tensor(name, list(shape), dtype, kind="ExternalInput")

    emb_d = din("emb", (V, DW), F32)
    idx_d = din("idx", (ROWS,), I32)
    encvT_d = din("encvT", (HDC, 128, SV * BC), F32)
    encvTb_d = din("encvTb", (HDC, 128, SV * BC), BF16)
    encpT_d = din("encpT", (PDC, 128, SP * BC), F32)
    encpTb_d = din("encpTb", (PDC, 128, SP * BC), BF16)
    Wah_d = din("Wah", (8, 128, 768), BF16)
    avWe_d = din("avWe", (HDC, 128, H), BF16)
    apWe_d = din("apWe", (PDC, 128, PP), BF16)
    w2v_d = din("w2v", (HDC, 128, 1), BF16)
    w2p_d = din("w2p", (PDC, 128, 1), BF16)
    b1v_d = din("b1v", (HDC, 128, 1), F32)
    b1p_d = din("b1p", (PDC, 128, 1), F32)
    ihW0x_d = din("ihW0x", (HDC, 128, 2048), BF16)
    ihW0xm_d = din("ihW0xm", (HDC, 128, 32), BF16)
    ihW0c_d = din("ihW0c", (HDC, 128, 2048), BF16)
    ihW0cm_d = din("ihW0cm", (HDC, 128, 32), BF16)
    hhW0_d = din("hhW0", (HDC, 128, 2048), BF16)
    hhW0m_d = din("hhW0m", (HDC, 128, 32), BF16)
    ihW1_d = din("ihW1", (HDC, 128, 2048), BF16)
    ihW1m_d = din("ihW1m", (HDC, 128, 32), BF16)
    hhW1_d = din("hhW1", (HDC, 128, 2048), BF16)
    hhW1m_d = din("hhW1m", (HDC, 128, 32), BF16)
    phW0_d = din("phW0", (PDC, 128, 32), BF16)
    phW1_d = din("phW1", (PDC, 128, 32), BF16)
    bg0_d = din("bg0", (128, NGT), F32)
    bg1_d = din("bg1", (128, NGT), F32)
    bm0_d = din("bm0", (1, 32), F32)
    bm1_d = din("bm1", (1, 32), F32)
    Ecin_d = din("Ecin", (HDC, 32, 128), F32)
    Ecf_d = din("Ecf", (HDC, 32, 128), F32)
    L32_d = din("L32", (32, 32), F32)
    E2_d = din("E2", (2, 32), F32)
    E2T_d = din("E2T", (32, 2), F32)
    outW_d = din("outW", (NVC, HDC, 128, 512), FP8)

    out_d = nc.dram_tensor("out", [ROWS, VPAD], BF16, kind="ExternalOutput")

    with TileContext(nc) as tc:
        with (
            tc.tile_pool(name="consts", bufs=1) as consts,
            tc.tile_pool(name="keep", bufs=1) as keep,
        ):
            # ---------------- constants ----------------
            id_bf = consts.tile([128, 128], BF16)
            make_identity(nc, id_bf)
            ones_bf = consts.tile([1, 128], BF16)
            nc.gpsimd.memset(ones_bf, 1.0)
            ones_f = consts.tile([1, ROWS], F32)
            nc.gpsimd.memset(ones_f, 1.0)
            Ecin = consts.tile([32, HDC, 128], F32)
            Ecf = consts.tile([32, HDC, 128], F32)
            for c in range(HDC):
                nc.sync.dma_start(out=Ecin[:, c], in_=Ecin_d[c])
                nc.sync.dma_start(out=Ecf[:, c], in_=Ecf_d[c])
            L32 = consts.tile([32, 32], F32)
            nc.sync.dma_start(out=L32, in_=L32_d[:, :])
            E2 = consts.tile([2, 32], F32)
            nc.sync.dma_start(out=E2, in_=E2_d[:, :])
            E2T = consts.tile([32, 2], F32)
            nc.sync.dma_start(out=E2T, in_=E2T_d[:, :])
            bg0 = consts.tile([128, NGT], F32)
            bg1 = consts.tile([128, NGT], F32)
            nc.sync.dma_start(out=bg0, in_=bg0_d[:, :])
            nc.sync.dma_start(out=bg1, in_=bg1_d[:, :])
            bm0 = consts.tile([1, 32], F32)
            bm1 = consts.tile([1, 32], F32)
            nc.sync.dma_start(out=bm0, in_=bm0_d[:, :])
            nc.sync.dma_start(out=bm1, in_=bm1_d[:, :])
            w2v = consts.tile([128, HDC, 1], BF16)
            w2p = consts.tile([128, PDC, 1], BF16)
            b1v = consts.tile([128, HDC, 1], F32)
            b1p = consts.tile([128, PDC, 1], F32)
            for c in range(HDC):
                nc.sync.dma_start(out=w2v[:, c], in_=w2v_d[c])
                nc.sync.dma_start(out=b1v[:, c], in_=b1v_d[c])
            for c in range(PDC):
                nc.sync.dma_start(out=w2p[:, c], in_=w2p_d[c])
                nc.sync.dma_start(out=b1p[:, c], in_=b1p_d[c])

            # h1 for all steps (fp8, pre-scaled by H1SC) — projection lhsT
            h1_all = keep.tile([128, HDC, NS, BC], FP8)

            if flags.get("skip_recur"):
                nc.gpsimd.memset(h1_all, 0.0)
            # ================= recurrence scope =================
            if not flags.get("skip_recur"):
                with (
                    tc.tile_pool(name="rkeep", bufs=1) as rk,
                    tc.tile_pool(name="states", bufs=3) as stp,
                    tc.tile_pool(name="wk", bufs=2) as wk,
                    tc.tile_pool(name="wkbig", bufs=1) as wkb,
                ):
                    def wload(pool, dram, kdim, n, nm, dt=BF16):
                        t = pool.tile([128, kdim, n], dt, name=nm, tag=nm, bufs=1)
                        for c in range(kdim):
                            nc.sync.dma_start(out=t[:, c], in_=dram[c])
                        return t

                    encvTb = rk.tile([128, HDC, SV * BC], BF16)
                    encpTb = rk.tile([128, PDC, SP * BC], BF16)
                    for c in range(HDC):
                        nc.sync.dma_start(out=encvTb[:, c], in_=encvTb_d[c])
                    for c in range(PDC):
                        nc.sync.dma_start(out=encpTb[:, c], in_=encpTb_d[c])

                    stat0 = rk.tile([128, NGT, ROWS], BF16)
                    m0stat = rk.tile([32, NS, BC], F32)
                    encWv = rk.tile([128, HDC, SV * BC], BF16)
                    encWp = rk.tile([128, PDC, SP * BC], BF16)

                    # ---- preamble ----
                    with (
                        tc.tile_pool(name="pre", bufs=2) as pre,
                        tc.tile_pool(name="ppre", bufs=1, space="PSUM") as ppre,
                    ):
                        avWe = wload(pre, avWe_d, HDC, H, "avWe")
                        apWe = wload(pre, apWe_d, PDC, PP, "apWe")
                        ihW0x = wload(pre, ihW0x_d, HDC, 2048, "ihW0x")
                        ihW0xm = wload(pre, ihW0xm_d, HDC, 32, "ihW0xm")

                        NTI = (ROWS + 127) // 128
                        idx_sb = pre.tile([128, NTI], I32, tag="idx")
                        nfull = ROWS // 128
                        if nfull:
                            nc.sync.dma_start(
                                out=idx_sb[:, :nfull],
                                in_=idx_d[: nfull * 128].rearrange("(i p) -> p i", p=128),
                            )
                        if ROWS % 128:
                            nc.sync.dma_start(
                                out=idx_sb[: ROWS % 128, nfull : nfull + 1],
                                in_=idx_d[nfull * 128 :],
                            )
                        embT = pre.tile([128, HDC, ROWS], BF16, tag="embT")
                        for i in range(NTI):
                            n = min(128, ROWS - i * 128)
                            esb = pre.tile([128, DW], F32, tag="esb")
                            nc.gpsimd.indirect_dma_start(
                                out=esb[:n],
                                out_offset=None,
                                in_=emb_d[:, :],
                                in_offset=IndirectOffsetOnAxis(
                                    ap=idx_sb[:n, i : i + 1], axis=0
                                ),
                            )
                            ebf = pre.tile([128, DW], BF16, tag="ebf")
                            nc.vector.tensor_copy(out=ebf[:n], in_=esb[:n])
                            for c in range(HDC):
                                tp = ppre.tile([128, 128], BF16, tag="tp")
                                nc.tensor.transpose(
                                    tp[:, :n],
                                    ebf[:n, c * 128 : (c + 1) * 128],
                                    id_bf[:n, :n],
                                )
                                nc.vector.tensor_copy(
                                    out=embT[:, c, i * 128 : i * 128 + n], in_=tp[:, :n]
                                )

                        # static gate part from xt: stat0 = ihW0x.T @ embT (+bias_g0)
                        for gt in range(NGT):
                            sp = ppre.tile([128, ROWS], F32, tag="sp")
                            for c in range(HDC):
                                nc.tensor.matmul(
                                    sp,
                                    ihW0x[:, c, gt * 128 : (gt + 1) * 128],
                                    embT[:, c],
                                    start=(c == 0),
                                    stop=(c == HDC - 1),
                                )
                            if flags["bg0_nz"]:
                                nc.vector.tensor_tensor(
                                    out=stat0[:, gt], in0=sp,
                                    in1=bg0[:, gt : gt + 1].to_broadcast([128, ROWS]),
                                    op=ALU.add,
                                )
                            else:
                                nc.vector.tensor_copy(out=stat0[:, gt], in_=sp)
                        # static master part (transposed): ihW0xm.T @ embT + bm0
                        mp = ppre.tile([32, ROWS], F32, tag="mp")
                        for c in range(HDC):
                            nc.tensor.matmul(
                                mp,
                                ihW0xm[:, c],
                                embT[:, c],
                                start=(c == 0),
                                stop=(c == HDC - 1 and not flags["bm0_nz"]),
                            )
                        if flags["bm0_nz"]:
                            nc.tensor.matmul(mp, bm0, ones_f, start=False, stop=True)
                        nc.vector.tensor_copy(
                            out=m0stat.rearrange("p t b -> p (t b)"), in_=mp
                        )

                        # encoder attention precompute (enc @ W1_enc + b1), transposed
                        for m in range(HDC):
                            ep = ppre.tile([128, 2, 512], F32, tag="ep")
                            for hh in range(2):
                                for c in range(HDC):
                                    nc.tensor.matmul(
                                        ep[:, hh, :320],
                                        avWe[:, c, m * 128 : (m + 1) * 128],
                                        encvTb[:, c, hh * 320 : (hh + 1) * 320],
                                        start=(c == 0),
                                        stop=(c == HDC - 1),
                                    )
                                if flags["b1v_nz"]:
                                    nc.vector.tensor_tensor(
                                        out=encWv[:, m, hh * 320 : (hh + 1) * 320],
                                        in0=ep[:, hh, :320],
                                        in1=b1v[:, m].to_broadcast([128, 320]),
                                        op=ALU.add,
                                    )
                                else:
                                    nc.vector.tensor_copy(
                                        out=encWv[:, m, hh * 320 : (hh + 1) * 320],
                                        in_=ep[:, hh, :320],
                                    )
                        for m in range(PDC):
                            ep2 = ppre.tile([128, SP * BC], F32, tag="ep2")
                            for c in range(PDC):
                                nc.tensor.matmul(
                                    ep2,
                                    apWe[:, c, m * 128 : (m + 1) * 128],
                                    encpTb[:, c],
                                    start=(c == 0),
                                    stop=(c == PDC - 1),
                                )
                            if flags["b1p_nz"]:
                                nc.vector.tensor_tensor(
                                    out=encWp[:, m], in0=ep2,
                                    in1=b1p[:, m].to_broadcast([128, SP * BC]),
                                    op=ALU.add,
                                )
                            else:
                                nc.vector.tensor_copy(out=encWp[:, m], in_=ep2)

                    # ---- states ----
                    h0T = stp.tile([128, HDC, BC], F32, tag="hn0")
                    c0T = stp.tile([128, HDC, BC], F32, tag="cn0")
                    h1T = stp.tile([128, HDC, BC], F32, tag="hn1")
                    c1T = stp.tile([128, HDC, BC], F32, tag="cn1")
                    for s in (h0T, c0T, h1T, c1T):
                        nc.gpsimd.memset(s, 0.0)

                    with (
                        tc.tile_pool(name="wpool", bufs=1) as wp,
                        tc.tile_pool(name="pstep", bufs=1, space="PSUM") as pst,
                    ):
                        Wah = wload(wp, Wah_d, 8, 768, "Wah")
                        ihW0c = wload(wp, ihW0c_d, HDC, 2048, "ihW0c")
                        hhW0 = wload(wp, hhW0_d, HDC, 2048, "hhW0")
                        ihW1 = wload(wp, ihW1_d, HDC, 2048, "ihW1")
                        hhW1 = wload(wp, hhW1_d, HDC, 2048, "hhW1")
                        ihW0cm = wload(wp, ihW0cm_d, HDC, 32, "ihW0cm")
                        hhW0m = wload(wp, hhW0m_d, HDC, 32, "hhW0m")
                        ihW1m = wload(wp, ihW1m_d, HDC, 32, "ihW1m")
                        hhW1m = wload(wp, hhW1m_d, HDC, 32, "hhW1m")
                        phW0 = wload(wp, phW0_d, PDC, 32, "phW0")
                        phW1 = wload(wp, phW1_d, PDC, 32, "phW1")

                        def attend(hidS, hid_off, ndc, S, encWb, encTb, w2, tag):
                            nb = S * BC
                            nh = (nb + 511) // 512
                            half = (nb + nh - 1) // nh
                            tz = wkb.tile([128, ndc, nb], BF16, tag=f"tz{tag}")
                            for c in range(ndc):
                                for hh in range(nh):
                                    lo, hi = hh * half, min((hh + 1) * half, nb)
                                    ns = (hi - lo) // BC
                                    zc = pst.tile([128, 512], F32, tag="z", bufs=2)
                                    nc.tensor.matmul(
                                        zc[:, : hi - lo], id_bf,
                                        encWb[:, c, lo:hi], start=True, stop=False,
                                    )
                                    nc.tensor.matmul(
                                        zc[:, : hi - lo].rearrange(
                                            "p (s b) -> p s b", b=BC),
                                        id_bf,
                                        hidS[:, hid_off + c]
                                        .rearrange("p b -> p () b")
                                        .to_broadcast([128, ns, BC]),
                                        start=False, stop=True,
                                    )
                                    nc.scalar.activation(
                                        tz[:, c, lo:hi], zc[:, : hi - lo], AF.Tanh
                                    )
                            e_ps = pst.tile([1, nh, 512], F32, tag="e")
                            for hh in range(nh):
                                lo, hi = hh * half, min((hh + 1) * half, nb)
                                for c in range(ndc):
                                    nc.tensor.matmul(
                                        e_ps[:, hh, : hi - lo],
                                        w2[:, c],
                                        tz[:, c, lo:hi],
                                        start=(c == 0),
                                        stop=(c == ndc - 1),
                                    )
                            aexp = wk.tile([1, nb], BF16, tag=f"ax{tag}")
                            nc.scalar.activation(
                                aexp.rearrange("o (h x) -> o h x", h=nh),
                                e_ps[:, :, :half],
                                AF.Exp,
                            )
                            # unnormalized context; 1/sum folded in at the end
                            ssum = wk.tile([1, BC], F32, tag=f"ss{tag}")
                            nc.vector.tensor_reduce(
                                out=ssum,
                                in_=aexp.rearrange("o (s b) -> o b s", b=BC),
                                axis=AX.X,
                                op=ALU.add,
                            )
                            rec = wk.tile([1, BC], F32, tag=f"rc{tag}")
                            nc.vector.reciprocal(rec, ssum)
                            rrep = pst.tile([128, 512], F32, tag="z", bufs=2)
                            nc.tensor.matmul(
                                rrep[:, :BC], ones_f[:, :128], rec,
                                start=True, stop=True,
                            )
                            arep = pst.tile([128, nh, 512], F32, tag="e")
                            for hh in range(nh):
                                lo, hi = hh * half, min((hh + 1) * half, nb)
                                nc.tensor.matmul(
                                    arep[:, hh, : hi - lo],
                                    ones_bf,
                                    aexp[:, lo:hi],
                                    start=True,
                                    stop=True,
                                )
                            cvT = wk.tile([128, ndc, BC], F32, tag=f"cv{tag}")
                            prod = wkb.tile([128, ndc, nb], F32, tag=f"pr{tag}")
                            for hh in range(nh):
                                lo, hi = hh * half, min((hh + 1) * half, nb)
                                nc.vector.tensor_tensor(
                                    out=prod[:, :, lo:hi],
                                    in0=encTb[:, :, lo:hi],
                                    in1=arep[:, hh, : hi - lo]
                                    .rearrange("p x -> p () x")
                                    .to_broadcast([128, ndc, hi - lo]),
                                    op=ALU.mult,
                                )
                            nc.vector.tensor_reduce(
                                out=cvT,
                                in_=prod.rearrange("p c (s b) -> p c b s", b=BC),
                                axis=AX.X,
                                op=ALU.add,
                            )
                            cvb = wk.tile([128, ndc, BC], BF16, tag=f"cb{tag}")
                            nc.vector.tensor_tensor(
                                out=cvb,
                                in0=cvT,
                                in1=rrep[:, :BC]
                                .rearrange("p b -> p () b")
                                .to_broadcast([128, ndc, BC]),
                                op=ALU.mult,
                            )
                            return cvb

                        def cumsoft_reps(mch, m_ps, mstat_ap, tag):
                            # m_ps: psum [32, BC] master logits (transposed)
                            if mstat_ap is not None:
                                ms = wk.tile([32, BC], F32, tag=f"ms{tag}")
                                nc.vector.tensor_tensor(
                                    out=ms, in0=m_ps, in1=mstat_ap, op=ALU.add
                                )
                                esrc = ms
                            else:
                                esrc = m_ps
                            em = wk.tile([32, BC], F32, tag=f"em{tag}")
                            nc.scalar.activation(em, esrc, AF.Exp)
                            cs = mch[:32, 2 * BC : 3 * BC]
                            nc.tensor.matmul(cs, L32, em, start=True, stop=True)
                            tot = mch[:2, 18 * BC : 19 * BC]
                            nc.tensor.matmul(tot, E2T, em, start=True, stop=True)
                            rec2 = wk.tile([2, BC], F32, tag=f"r2{tag}")
                            nc.vector.reciprocal(rec2, tot)
                            rr = mch[:32, 3 * BC : 4 * BC]
                            nc.tensor.matmul(rr, E2, rec2, start=True, stop=True)
                            rrS = wk.tile([32, BC], F32, tag=f"rrS{tag}")
                            nc.vector.tensor_copy(out=rrS, in_=rr)
                            csn = wk.tile([32, BC], F32, tag=f"cf{tag}")
                            nc.vector.tensor_tensor(
                                out=csn, in0=cs, in1=rrS, op=ALU.mult
                            )
                            ci32 = wk.tile([32, BC], F32, tag=f"ci{tag}")
                            nc.vector.tensor_scalar(
                                out=ci32, in0=csn, scalar1=-1.0, scalar2=1.0,
                                op0=ALU.mult, op1=ALU.add,
                            )
                            rep = mch[:, 4 * BC : 4 * BC + HDC * 2 * BC].rearrange(
                                "p (c a b) -> p c a b", c=HDC, a=2
                            )
                            for tau in range(HDC):
                                nc.tensor.matmul(
                                    rep[:, tau, 0], Ecin[:, tau], ci32,
                                    start=True, stop=True,
                                )
                                nc.tensor.matmul(
                                    rep[:, tau, 1], Ecf[:, tau], csn,
                                    start=True, stop=True,
                                )
                            repS = wk.tile([128, HDC, 2, BC], F32, tag=f"rs{tag}")
                            nc.vector.tensor_copy(out=repS, in_=rep)
                            return repS

                        def combine(ga, repS, cT, tag):
                            ci = repS[:, :, 0]
                            cf = repS[:, :, 1]
                            ov = wk.tile([128, HDC, BC], F32, tag=f"ov{tag}")
                            nc.vector.tensor_tensor(out=ov, in0=ci, in1=cf, op=ALU.mult)
                            fg_ = wk.tile([128, HDC, BC], F32, tag=f"fg{tag}")
                            ig_ = wk.tile([128, HDC, BC], F32, tag=f"ig{tag}")
                            tmp = wk.tile([128, HDC, BC], F32, tag=f"tm{tag}")
                            nc.vector.tensor_tensor(
                                out=fg_, in0=ga[:, 8:12], in1=ov, op=ALU.mult
                            )
                            nc.vector.tensor_tensor(
                                out=tmp, in0=cf, in1=ov, op=ALU.subtract
                            )
                            nc.vector.tensor_tensor(out=fg_, in0=fg_, in1=tmp, op=ALU.add)
                            nc.vector.tensor_tensor(
                                out=ig_, in0=ga[:, 4:8], in1=ov, op=ALU.mult
                            )
                            nc.vector.tensor_tensor(
                                out=tmp, in0=ci, in1=ov, op=ALU.subtract
                            )
                            nc.vector.tensor_tensor(out=ig_, in0=ig_, in1=tmp, op=ALU.add)
                            cn = stp.tile([128, HDC, BC], F32, tag=f"cn{tag}")
                            nc.vector.tensor_tensor(out=cn, in0=fg_, in1=cT, op=ALU.mult)
                            nc.vector.tensor_tensor(
                                out=tmp, in0=ig_, in1=ga[:, 12:16], op=ALU.mult
                            )
                            nc.vector.tensor_tensor(out=cn, in0=cn, in1=tmp, op=ALU.add)
                            tcy = wk.tile([128, HDC, BC], F32, tag=f"tc{tag}")
                            nc.scalar.activation(tcy, cn, AF.Tanh)
                            hn = stp.tile([128, HDC, BC], F32, tag=f"hn{tag}")
                            nc.vector.tensor_tensor(
                                out=hn, in0=ga[:, 0:4], in1=tcy, op=ALU.mult
                            )
                            return hn, cn

                        # ================= the 27 steps =================
                        MO_HID = 192          # free-offset of hid inside mch
                        for t in range(NS):
                            h0b = wk.tile([128, HDC, BC], BF16, tag="h0b")
                            h1b = wk.tile([128, HDC, BC], BF16, tag="h1b")
                            nc.vector.tensor_copy(out=h0b, in_=h0T)
                            nc.vector.tensor_copy(out=h1b, in_=h1T)
                            mch = pst.tile([128, 512], F32, tag="mch")
                            hid = mch[:, MO_HID : MO_HID + 6 * BC].rearrange(
                                "p (m b) -> p m b", b=BC
                            )
                            m0 = mch[:32, 0:BC]
                            m1 = mch[:32, BC : 2 * BC]
                            for mt in range(6):
                                for kc in range(8):
                                    rhs = h0b[:, kc] if kc < 4 else h1b[:, kc - 4]
                                    nc.tensor.matmul(
                                        hid[:, mt],
                                        Wah[:, kc, mt * 128 : (mt + 1) * 128],
                                        rhs,
                                        start=(kc == 0),
                                        stop=(kc == 7),
                                    )
                            # gate/master matmuls that depend only on prior state
                            # start their PSUM groups early (overlap attention)
                            g0 = pst.tile([128, NGT, BC], F32, tag="g", bufs=2)
                            g1 = pst.tile([128, NGT, BC], F32, tag="g", bufs=2)
                            for gt in range(NGT):
                                for kc in range(HDC):
                                    nc.tensor.matmul(
                                        g0[:, gt],
                                        hhW0[:, kc, gt * 128 : (gt + 1) * 128],
                                        h0b[:, kc],
                                        start=(kc == 0),
                                        stop=False,
                                    )
                            for gt in range(NGT):
                                for kc in range(HDC):
                                    nc.tensor.matmul(
                                        g1[:, gt],
                                        hhW1[:, kc, gt * 128 : (gt + 1) * 128],
                                        h1b[:, kc],
                                        start=(kc == 0),
                                        stop=False,
                                    )
                            for kc in range(HDC):
                                nc.tensor.matmul(
                                    m0, hhW0m[:, kc], h0b[:, kc],
                                    start=(kc == 0), stop=False,
                                )
                            for kc in range(HDC):
                                nc.tensor.matmul(
                                    m1, hhW1m[:, kc], h1b[:, kc],
                                    start=(kc == 0), stop=False,
                                )
                            hidS = wk.tile([128, 6, BC], BF16, tag="hidS")
                            nc.vector.tensor_copy(out=hidS, in_=hid)
                            cvb = attend(hidS, 0, HDC, SV, encWv, encvTb, w2v, "v")
                            cpb = attend(hidS, 4, PDC, SP, encWp, encpTb, w2p, "p")

                            # ---- layer 0 finish ----
                            for kc in range(PDC):
                                nc.tensor.matmul(
                                    m0, phW0[:, kc], cpb[:, kc],
                                    start=False, stop=False,
                                )
                                nc.tensor.matmul(
                                    m1, phW1[:, kc], cpb[:, kc],
                                    start=False, stop=False,
                                )
                            for kc in range(HDC):
                                nc.tensor.matmul(
                                    m0, ihW0cm[:, kc], cvb[:, kc],
                                    start=False, stop=(kc == HDC - 1),
                                )
                            for gt in range(NGT):
                                for kc in range(HDC):
                                    nc.tensor.matmul(
                                        g0[:, gt],
                                        ihW0c[:, kc, gt * 128 : (gt + 1) * 128],
                                        cvb[:, kc],
                                        start=False,
                                        stop=(kc == HDC - 1),
                                    )
                            rep0 = cumsoft_reps(mch, m0, m0stat[:, t], "0")
                            g0s = wk.tile([128, NGT, BC], F32, tag="g0s")
                            nc.vector.tensor_tensor(
                                out=g0s,
                                in0=g0,
                                in1=stat0.rearrange("p g (t b) -> p g t b", b=BC)[:, :, t],
                                op=ALU.add,
                            )
                            g0a = wk.tile([128, NGT, BC], F32, tag="g0a")
                            g0t = wk.tile([128, 12, BC], F32, tag="g0t")
                            nc.scalar.activation(g0t, g0s[:, 0:12], AF.Tanh, scale=0.5)
                            nc.scalar.activation(g0a[:, 12:16], g0s[:, 12:16], AF.Tanh)
                            nc.vector.tensor_scalar(
                                out=g0a[:, 0:12], in0=g0t, scalar1=0.5,
                                scalar2=0.5, op0=ALU.mult, op1=ALU.add)
                            h0T, c0T = combine(g0a, rep0, c0T, "0")

                            # ---- layer 1 finish ----
                            h0b2 = wk.tile([128, HDC, BC], BF16, tag="h0b2")
                            nc.vector.tensor_copy(out=h0b2, in_=h0T)
                            for kc in range(HDC):
                                nc.tensor.matmul(
                                    m1, ihW1m[:, kc], h0b2[:, kc],
                                    start=False,
                                    stop=(kc == HDC - 1 and not flags["bm1_nz"]),
                                )
                            if flags["bm1_nz"]:
                                nc.tensor.matmul(
                                    m1, bm1, ones_f[:, :BC], start=False, stop=True
                                )
                            for gt in range(NGT):
                                for kc in range(HDC):
                                    nc.tensor.matmul(
                                        g1[:, gt],
                                        ihW1[:, kc, gt * 128 : (gt + 1) * 128],
                                        h0b2[:, kc],
                                        start=False,
                                        stop=(kc == HDC - 1),
                                    )
                            rep1 = cumsoft_reps(mch, m1, None, "1")
                            g1a = wk.tile([128, NGT, BC], F32, tag="g1a")
                            if flags["bg1_nz"]:
                                g1s = wk.tile([128, NGT, BC], F32, tag="g1s")
                                for gt in range(NGT):
                                    nc.vector.tensor_tensor(
                                        out=g1s[:, gt],
                                        in0=g1[:, gt],
                                        in1=bg1[:, gt : gt + 1].to_broadcast([128, BC]),
                                        op=ALU.add,
                                    )
                                gsrc = g1s
                            else:
                                gsrc = g1
                            g1t = wk.tile([128, 12, BC], F32, tag="g1t")
                            nc.scalar.activation(g1t, gsrc[:, 0:12], AF.Tanh, scale=0.5)
                            nc.scalar.activation(g1a[:, 12:16], gsrc[:, 12:16], AF.Tanh)
                            nc.vector.tensor_scalar(
                                out=g1a[:, 0:12], in0=g1t, scalar1=0.5,
                                scalar2=0.5, op0=ALU.mult, op1=ALU.add)
                            h1T, c1T = combine(g1a, rep1, c1T, "1")
                            nc.vector.tensor_scalar(
                                out=h1_all[:, :, t], in0=h1T, scalar1=H1SC,
                                scalar2=None, op0=ALU.mult,
                            )

            # ================= projection =================
            # Two fp8 DoubleRow matmul passes per m-tile, no z storage:
            #   A(m): z = h1fp8.T @ Wfp8 -> exp(z*ZSC) (throwaway fp8 out)
            #         with accum_out building S per row
            #   B(m): recompute z, lp = z*ZSC + ln(1/S) via DVE/Pool
            #         tensor_scalar, bf16 out -> DMA
            # B(m) runs on DVE/Pool/DMA while A(m+1) occupies Act -> pipeline.
            if not flags.get("skip_proj"):
                NAG = (NVC + AGRP - 1) // AGRP  # 20 pass-A groups
                BGRP = 8                       # pass-B chunks per out DMA
                with (
                    tc.tile_pool(name="pj", bufs=1) as pj,
                    tc.tile_pool(name="lpout", bufs=3) as lpo,
                    tc.tile_pool(name="ppa", bufs=2, space="PSUM") as ppa,
                    tc.tile_pool(name="ppb", bufs=2, space="PSUM") as ppb,
                ):
                    wtall = pj.tile([128, NVC, HDC, 512], FP8)
                    Sacc = pj.tile([128, NM, NAG], F32)
                    scr = pj.tile([128, AGRP * 512], FP8)
                    neglogS = pj.tile([128, NM], F32)
                    recS = pj.tile([128, NM], F32)
                    StotM = pj.tile([128, NM], F32)
                    h1f = h1_all.rearrange("p c t b -> p c (t b)")

                    loaded = [0]  # wtall chunks DMA'd so far

                    def wt_load_until(v_needed):
                        while loaded[0] <= min(v_needed + 5, NVC - 1):
                            v = loaded[0]
                            nc.sync.dma_start(
                                out=wtall[:, v],
                                in_=outW_d[v].rearrange("c p n -> p c n"),
                            )
                            loaded[0] += 1

                    pool_turn = [0]

                    for m in range(NM):
                        nr = M_ROWS[m]
                        rows = slice(m * 128, m * 128 + nr)
                        # ---- pass A: exp + accumulate S ----
                        for g in range(NAG):
                            v0 = g * AGRP
                            nv = min(AGRP, NVC - v0)
                            if m == 0:
                                wt_load_until(v0 + nv - 1)
                            ps = ppa.tile([128, AGRP * 512], F32, tag="pa")
                            for vv in range(nv):
                                for cp in range(HDC // 2):
                                    nc.tensor.matmul(
                                        ps[:nr, vv * 512 : (vv + 1) * 512],
                                        h1f[:, 2 * cp : 2 * cp + 2, rows],
                                        wtall[:, v0 + vv, 2 * cp : 2 * cp + 2],
                                        start=(cp == 0),
                                        stop=(cp == HDC // 2 - 1),
                                        perf_mode=DR,
                                    )
                            nc.scalar.activation(
                                scr[:nr, : nv * 512],
                                ps[:nr, : nv * 512],
                                AF.Exp,
                                scale=ZSC,
                                accum_out=Sacc[:nr, m, g : g + 1],
                            )
                        nc.vector.tensor_reduce(
                            out=StotM[:nr, m : m + 1], in_=Sacc[:nr, m],
                            axis=AX.X, op=ALU.add,
                        )
                        nc.vector.reciprocal(recS[:nr, m : m + 1], StotM[:nr, m : m + 1])
                        nc.scalar.activation(
                            neglogS[:nr, m : m + 1], recS[:nr, m : m + 1], AF.Ln
                        )
                        # ---- pass B: recompute z, subtract logS, write out ----
                        for v0 in range(0, NVC, BGRP):
                            nv = min(BGRP, NVC - v0)
                            lp = lpo.tile([128, BGRP * 512], BF16, tag="lp")
                            for vv in range(nv):
                                v = v0 + vv
                                ps = ppb.tile([128, 512], F32, tag="pb")
                                for cp in range(HDC // 2):
                                    nc.tensor.matmul(
                                        ps[:nr],
                                        h1f[:, 2 * cp : 2 * cp + 2, rows],
                                        wtall[:, v, 2 * cp : 2 * cp + 2],
                                        start=(cp == 0),
                                        stop=(cp == HDC // 2 - 1),
                                        perf_mode=DR,
                                    )
                                pool_turn[0] += 1
                                if m == NM - 1 and pool_turn[0] % 2 == 0:
                                    # Act is free after the last pass A
                                    nc.scalar.activation(
                                        out=lp[:nr, vv * 512 : (vv + 1) * 512],
                                        in_=ps[:nr],
                                        func=AF.Identity,
                                        bias=neglogS[:nr, m : m + 1],
                                        scale=ZSC,
                                    )
                                else:
                                    nc.vector.tensor_scalar(
                                        out=lp[:nr, vv * 512 : (vv + 1) * 512],
                                        in0=ps[:nr],
                                        scalar1=ZSC,
                                        scalar2=neglogS[:nr, m : m + 1],
                                        op0=ALU.mult,
                                        op1=ALU.add,
                                    )
                            nc.sync.dma_start(
                                out=out_d[rows, v0 * 512 : (v0 + nv) * 512],
                                in_=lp[:nr, : nv * 512],
                            )



    nc.finalize()
    return nc


def _prep(inputs):
    """Host-side input prep: slicing/transposing/casting only."""
    f32 = np.float32
    g = {k: np.asarray(v) for k, v in inputs.items()}
    av_W1, ap_W1 = g["av_W1"].astype(f32), g["ap_W1"].astype(f32)
    shared = {}
    shared["emb"] = np.ascontiguousarray(g["embedding"].astype(f32))
    shared["Wah"] = np.ascontiguousarray(
        np.concatenate([av_W1[H:], ap_W1[PP:]], axis=1).reshape(8, 128, 768)
    ).astype(BF)
    shared["avWe"] = np.ascontiguousarray(av_W1[:H].reshape(HDC, 128, H)).astype(BF)
    shared["apWe"] = np.ascontiguousarray(ap_W1[:PP].reshape(PDC, 128, PP)).astype(BF)
    shared["w2v"] = g["av_w2"].astype(f32).reshape(HDC, 128, 1).astype(BF)
    shared["w2p"] = g["ap_w2"].astype(f32).reshape(PDC, 128, 1).astype(BF)
    shared["b1v"] = np.ascontiguousarray(g["av_b1"].astype(f32).reshape(HDC, 128, 1))
    shared["b1p"] = np.ascontiguousarray(g["ap_b1"].astype(f32).reshape(PDC, 128, 1))

    def gperm(Wg):
        # reference gate col order [outg|cellg|ing|fg] -> [outg|ing|fg|cellg]
        return np.concatenate(
            [Wg[..., 0:512], Wg[..., 1024:2048], Wg[..., 512:1024]], axis=-1)

    def cellw(W, kdim, pref):
        W = np.asarray(W, f32)
        return {
            pref: np.ascontiguousarray(
                gperm(W[:, 32:]).reshape(kdim, 128, 2048)).astype(BF),
            pref + "m": np.ascontiguousarray(W[:, :32].reshape(kdim, 128, 32)).astype(BF),
        }

    shared.update(cellw(g["ih_W0"][:DW], HDC, "ihW0x"))
    shared.update(cellw(g["ih_W0"][DW:], HDC, "ihW0c"))
    shared.update(cellw(g["hh_W0"], HDC, "hhW0"))
    shared.update(cellw(g["ih_W1"], HDC, "ihW1"))
    shared.update(cellw(g["hh_W1"], HDC, "hhW1"))
    shared["phW0"] = np.ascontiguousarray(
        g["ph_W0"].astype(f32).reshape(PDC, 128, 32)).astype(BF)
    shared["phW1"] = np.ascontiguousarray(
        g["ph_W1"].astype(f32).reshape(PDC, 128, 32)).astype(BF)
    bg0 = gperm((g["ih_b0"] + g["hh_b0"]).astype(f32)[32:])
    bg1 = gperm((g["ih_b1"] + g["hh_b1"]).astype(f32)[32:])
    shared["bg0"] = np.ascontiguousarray(bg0.reshape(NGT, 128).T)
    shared["bg1"] = np.ascontiguousarray(bg1.reshape(NGT, 128).T)
    bm0 = (g["ih_b0"][:32] + g["hh_b0"][:32] + g["ph_b0"]).astype(f32)
    bm1 = (g["ih_b1"][:32] + g["hh_b1"][:32] + g["ph_b1"]).astype(f32)
    shared["bm0"] = np.ascontiguousarray(bm0.reshape(1, 32))
    shared["bm1"] = np.ascontiguousarray(bm1.reshape(1, 32))
    Ecin = np.zeros((HDC, 32, 128), f32)
    Ecf = np.zeros((HDC, 32, 128), f32)
    for tau in range(HDC):
        for mcol in range(128):
            c = (tau * 128 + mcol) // CH
            Ecin[tau, c, mcol] = 1.0
            Ecf[tau, NCH + c, mcol] = 1.0
    shared["Ecin"] = Ecin
    shared["Ecf"] = Ecf
    L32 = np.zeros((32, 32), f32)
    for k in range(32):
        for m2 in range(32):
            if k // NCH == m2 // NCH and k % NCH <= m2 % NCH:
                L32[k, m2] = 1.0
    shared["L32"] = L32
    E2 = np.zeros((2, 32), f32)
    E2[0, :NCH] = 1.0
    E2[1, NCH:] = 1.0
    shared["E2"] = E2
    shared["E2T"] = np.ascontiguousarray(E2.T)
    oW = np.zeros((DW, VPAD), f32)
    oW[:, :V] = g["out_W"].astype(f32) * WSC
    shared["outW"] = np.ascontiguousarray(
        oW.reshape(HDC, 128, NVC, 512).transpose(2, 0, 1, 3)
    ).astype(ml_dtypes.float8_e4m3)

    flags = {
        "bg0_nz": bool(np.any(bg0 != 0)),
        "b1v_nz": bool(np.any(np.asarray(g["av_b1"]) != 0)),
        "b1p_nz": bool(np.any(np.asarray(g["ap_b1"]) != 0)),
        "bg1_nz": bool(np.any(bg1 != 0)),
        "bm0_nz": bool(np.any(bm0 != 0)),
        "bm1_nz": bool(np.any(bm1 != 0)),
        "outb_nz": bool(np.any(np.asarray(g["out_b"]) != 0)),
    }
    if flags["outb_nz"]:
        raise NotImplementedError("nonzero out_b path not wired")

    in_maps = []
    targets = np.asarray(g["targets"])
    enc_v = np.asarray(g["encoder_outputs"], f32)
    enc_p = np.asarray(g["encoder_outputs_parse"], f32)
    for r in range(8):
        m = dict(shared)
        sl = slice(BC * r, BC * (r + 1))
        m["idx"] = np.ascontiguousarray(
            targets[sl, :NS].T.reshape(-1).astype(np.int32))
        evT = np.ascontiguousarray(
            enc_v[sl].transpose(2, 1, 0).reshape(HDC, 128, SV * BC))
        epT = np.ascontiguousarray(
            enc_p[sl].transpose(2, 1, 0).reshape(PDC, 128, SP * BC))
        m["encvT"] = evT
        m["encvTb"] = evT.astype(BF)
        m["encpT"] = epT
        m["encpTb"] = epT.astype(BF)
        in_maps.append(m)
    return in_maps, flags


def kernel(**inputs):
    in_maps, flags = _prep(inputs)
    nc = _build(flags)
    res = run_bass_kernel_spmd(nc, in_maps, core_ids=list(range(8)))
    outs = []
    for r in range(8):
        o = np.asarray(res.results[r]["out"])[:, :V].astype(np.float32)
        outs.append(o.reshape(NS, BC, V).transpose(1, 0, 2))
    return np.ascontiguousarray(np.concatenate(outs, axis=0))

